# revision 3
# baseline (speedup 1.0000x reference)
"""Multi-head attention (B=4, S=2048, D=1024, H=16, causal) on 8 TRN2 NeuronCores.

Sharding: core i handles batch i//2 and head-group i%2 (8 heads / 512 projection
columns). Each core computes a partial output projection over its 512 rows of Wo;
the host sums the two partials per batch and adds bo. No device collectives.
"""

import sys

for _p in ("/opt/trn_rl_repo",):
    if _p not in sys.path:
        sys.path.insert(0, _p)

import numpy as np
import ml_dtypes

BF16 = ml_dtypes.bfloat16

B, S, D = 4, 2048, 1024
H, HD = 16, 64
HPC = H // 2          # heads per core: 8
DPC = D // 2          # projection cols per core: 512
NCORES = 8
SCALE = 1.0 / np.sqrt(np.float32(HD))

_compiled = None


def _build():
    import concourse.bacc as bacc
    import concourse.mybir as mybir
    import concourse.tile as tile

    f32 = mybir.dt.float32
    bf = mybir.dt.bfloat16
    Exp = mybir.ActivationFunctionType.Exp

    nc = bacc.Bacc("TRN2", target_bir_lowering=False, debug=False)

    # DRAM I/O (per-core shard shapes)
    xtq = nc.dram_tensor("xtq", [D, S], bf, kind="ExternalInput")
    xtk = nc.dram_tensor("xtk", [D, S], bf, kind="ExternalInput")
    xtv = nc.dram_tensor("xtv", [D, S], bf, kind="ExternalInput")
    wq = nc.dram_tensor("wq", [D, DPC], bf, kind="ExternalInput")
    wk = nc.dram_tensor("wk", [D, DPC], bf, kind="ExternalInput")
    wv = nc.dram_tensor("wv", [D, DPC], bf, kind="ExternalInput")
    wo = nc.dram_tensor("wo", [DPC, D], bf, kind="ExternalInput")
    bq = nc.dram_tensor("bq", [1, DPC], bf, kind="ExternalInput")
    bk = nc.dram_tensor("bk", [1, DPC], bf, kind="ExternalInput")
    bv = nc.dram_tensor("bv", [1, DPC], bf, kind="ExternalInput")
    trimask = nc.dram_tensor("trimask", [128, 128], bf, kind="ExternalInput")
    y = nc.dram_tensor("y", [S, D], f32, kind="ExternalOutput")

    NKD = D // 128        # 8 contraction tiles for projections
    NST = S // 128        # 16 seq tiles
    NSB = S // 512        # 4 seq blocks
    NHP = HPC // 2        # 4 head pairs / 128-wide col groups

    with tile.TileContext(nc) as tc:
        with (
            tc.tile_pool(name="consts", bufs=1) as consts,
            tc.tile_pool(name="wqp", bufs=NKD) as wqp,
            tc.tile_pool(name="wkp", bufs=NKD) as wkp,
            tc.tile_pool(name="wvp", bufs=NKD) as wvp,
            tc.tile_pool(name="wop", bufs=4) as wop,
            tc.tile_pool(name="xt", bufs=NKD) as xtp,
            tc.tile_pool(name="qt", bufs=NHP) as qtp,
            tc.tile_pool(name="kt", bufs=NHP) as ktp,
            tc.tile_pool(name="vp", bufs=NST) as vpool,
            tc.tile_pool(name="ex", bufs=3) as expool,
            tc.tile_pool(name="ot", bufs=NHP) as otp,
            tc.tile_pool(name="ys", bufs=3) as ysp,
            tc.tile_pool(name="rb", bufs=2) as rbp,
            tc.tile_pool(name="rc", bufs=2) as rcp,
            tc.tile_pool(name="ps", bufs=4, space="PSUM") as psp,
            tc.tile_pool(name="sc", bufs=1, space="PSUM") as scp,
        ):
            # constants
            tri = consts.tile([128, 128], bf, tag="tri")
            nc.sync.dma_start(tri[:], trimask.ap()[:])
            ones = consts.tile([1, 512], bf, tag="ones")
            nc.gpsimd.memset(ones[:], 1.0)
            bqt = consts.tile([1, DPC], bf, tag="bq")
            nc.sync.dma_start(bqt[:], bq.ap()[:])
            bkt = consts.tile([1, DPC], bf, tag="bk")
            nc.sync.dma_start(bkt[:], bk.ap()[:])
            bvt = consts.tile([1, DPC], bf, tag="bv")
            nc.sync.dma_start(bvt[:], bv.ap()[:])

            # weights
            wqt, wkt, wvt = [], [], []
            for kd in range(NKD):
                for lst, pool, t in ((wqt, wqp, wq), (wkt, wkp, wk), (wvt, wvp, wv)):
                    w = pool.tile([128, DPC], bf)
                    nc.sync.dma_start(w[:], t.ap()[kd * 128:(kd + 1) * 128, :])
                    lst.append(w)
            wot = []
            for hp in range(4):
                w = wop.tile([128, D], bf)
                nc.sync.dma_start(w[:], wo.ap()[hp * 128:(hp + 1) * 128, :])
                wot.append(w)

            # ---- QT / KT projections: QT[n, s] = sum_d W[d, n] * xT[d, s] + b[n]
            qts, kts = [], []
            for (src, wts, bias, dest_list, dest_pool) in (
                (xtq, wqt, bqt, qts, qtp),
                (xtk, wkt, bkt, kts, ktp),
            ):
                xts = []
                for kd in range(NKD):
                    xt = xtp.tile([128, S], bf)
                    nc.sync.dma_start(xt[:], src.ap()[kd * 128:(kd + 1) * 128, :])
                    xts.append(xt)
                for hp in range(NHP):
                    dt_tile = dest_pool.tile([128, S], bf)
                    dest_list.append(dt_tile)
                    for sb in range(NSB):
                        ps = psp.tile([128, 512], f32, tag="ps")
                        for kd in range(NKD):
                            nc.tensor.matmul(
                                ps[:],
                                wts[kd][:, hp * 128:(hp + 1) * 128],
                                xts[kd][:, sb * 512:(sb + 1) * 512],
                                start=(kd == 0), stop=False,
                            )
                        nc.tensor.matmul(
                            ps[:],
                            bias[0:1, hp * 128:(hp + 1) * 128],
                            ones[0:1, :],
                            start=False, stop=True,
                        )
                        nc.vector.tensor_copy(dt_tile[:, sb * 512:(sb + 1) * 512], ps[:])

            # ---- V projection (natural layout), interleaved [8 heads x 65] with ones col
            vts = []
            xts = []
            for kd in range(NKD):
                xt = xtp.tile([128, S], bf)
                nc.sync.dma_start(xt[:], xtv.ap()[kd * 128:(kd + 1) * 128, :])
                xts.append(xt)
            for st in range(NST):
                vt = vpool.tile([128, HPC * 65], bf)
                vts.append(vt)
                ps = psp.tile([128, 512], f32, tag="ps")
                for kd in range(NKD):
                    nc.tensor.matmul(
                        ps[:],
                        xts[kd][:, st * 128:(st + 1) * 128],
                        wvt[kd][:],
                        start=(kd == 0), stop=False,
                    )
                nc.tensor.matmul(ps[:], ones[0:1, 0:128], bvt[0:1, :],
                                 start=False, stop=True)
                v3 = vt[:].rearrange("p (h c) -> p h c", h=HPC, c=65)
                nc.vector.tensor_copy(
                    v3[:, :, 0:64],
                    ps[:].rearrange("p (h c) -> p h c", h=HPC, c=64),
                )
                nc.gpsimd.memset(v3[:, :, 64:65], 1.0)

            # ---- attention per head; scoresT [k, q] chunks of 4 k-tiles
            ots = [otp.tile([128, S], bf, name=f"ot{i}", tag="ot") for i in range(NHP)]
            for h in range(HPC):
                hp, sub = h // 2, h % 2
                base = sub * 64
                qt_h = qts[hp][base:base + 64, :]
                kt_h = kts[hp][base:base + 64, :]
                for j in range(NSB):
                    av = psp.tile([128, 512], f32, tag="ps")
                    nkt = 4 * (j + 1)
                    for c in range(j + 1):
                        sc = scp.tile([128, 2048], f32)
                        for r in range(4):
                            kti = c * 4 + r
                            nc.tensor.matmul(
                                sc[:, r * 512:(r + 1) * 512],
                                kt_h[:, kti * 128:(kti + 1) * 128],
                                qt_h[:, j * 512:(j + 1) * 512],
                                start=True, stop=True,
                            )
                        ex = expool.tile([128, 2048], bf)
                        nc.scalar.activation(ex[:], sc[:], Exp, scale=float(SCALE))
                        if c == j:  # diagonal chunk: causal mask
                            for r in range(4):
                                if r > 0:
                                    nc.gpsimd.memset(ex[:, r * 512:r * 512 + r * 128], 0.0)
                                dcol = r * 512 + r * 128
                                nc.vector.tensor_mul(
                                    ex[:, dcol:dcol + 128],
                                    ex[:, dcol:dcol + 128],
                                    tri[:],
                                )
                        for r in range(4):
                            kti = c * 4 + r
                            nc.tensor.matmul(
                                av[0:65, :],
                                vts[kti][:, h * 65:(h + 1) * 65],
                                ex[:, r * 512:(r + 1) * 512],
                                start=(kti == 0), stop=(kti == nkt - 1),
                            )
                    recip = rcp.tile([1, 512], f32)
                    nc.vector.reciprocal(recip[:], av[64:65, :])
                    rb = rbp.tile([64, 512], f32)
                    nc.gpsimd.partition_broadcast(rb[:], recip[:], channels=64)
                    nc.vector.tensor_mul(
                        ots[hp][base:base + 64, j * 512:(j + 1) * 512],
                        av[0:64, :],
                        rb[:],
                    )

            # ---- output projection: y[s, e] = sum_d outT[d, s] * Wo[d, e]
            for st in range(NST):
                for eb in range(2):
                    ps = psp.tile([128, 512], f32, tag="ps")
                    for hp in range(NHP):
                        nc.tensor.matmul(
                            ps[:],
                            ots[hp][:, st * 128:(st + 1) * 128],
                            wot[hp][:, eb * 512:(eb + 1) * 512],
                            start=(hp == 0), stop=(hp == NHP - 1),
                        )
                    ys = ysp.tile([128, 512], f32)
                    nc.vector.tensor_copy(ys[:], ps[:])
                    nc.sync.dma_start(
                        y.ap()[st * 128:(st + 1) * 128, eb * 512:(eb + 1) * 512],
                        ys[:],
                    )

    nc.compile()
    return nc


def _shard_inputs(q_in, k_in, v_in, Wq, bq, Wk, bk, Wv, bv, Wo, bo):
    tri = np.triu(np.ones((128, 128), np.float32)).astype(BF16)  # mask[k,q]=1 iff k<=q
    in_maps = []
    for core in range(NCORES):
        b, g = core // 2, core % 2
        cs = slice(g * DPC, (g + 1) * DPC)
        in_maps.append({
            "xtq": np.ascontiguousarray(q_in[b].T).astype(BF16),
            "xtk": np.ascontiguousarray(k_in[b].T).astype(BF16),
            "xtv": np.ascontiguousarray(v_in[b].T).astype(BF16),
            "wq": Wq[:, cs].astype(BF16),
            "wk": Wk[:, cs].astype(BF16),
            "wv": Wv[:, cs].astype(BF16),
            "wo": np.ascontiguousarray(Wo[cs, :]).astype(BF16),
            "bq": bq[cs].reshape(1, DPC).astype(BF16),
            "bk": bk[cs].reshape(1, DPC).astype(BF16),
            "bv": bv[cs].reshape(1, DPC).astype(BF16),
            "trimask": tri,
        })
    return in_maps


def kernel(q_in, k_in, v_in, Wq, bq, Wk, bk, Wv, bv, Wo, bo, _trace=False):
    from concourse.bass_utils import run_bass_kernel_spmd

    global _compiled
    if _compiled is None:
        _compiled = _build()

    args = [np.asarray(a, np.float32) for a in
            (q_in, k_in, v_in, Wq, bq, Wk, bk, Wv, bv, Wo, bo)]
    in_maps = _shard_inputs(*args)
    res = run_bass_kernel_spmd(
        _compiled, in_maps, core_ids=list(range(NCORES)), trace=_trace,
    )
    bo_f = args[10]
    out = np.empty((B, S, D), np.float32)
    for b in range(B):
        out[b] = res.results[2 * b]["y"] + res.results[2 * b + 1]["y"] + bo_f
    if _trace:
        kernel.last_results = res
    return out


# revision 6
# speedup vs baseline: 1.4113x; 1.4113x over previous
"""Multi-head attention (B=4, S=2048, D=1024, H=16, causal) on 8 TRN2 NeuronCores.

Sharding: core i handles batch i//2 and head-group i%2 (8 heads / 512 projection
columns). Each core computes a partial output projection over its 512 rows of Wo;
the host sums the two partials per batch and adds bo. No device collectives.

Per-core dataflow (bf16 matmuls, fp32 softmax):
  QT/KT = W-stationary projections of pre-transposed x; V in natural layout with
  an interleaved ones column per head (softmax denominator rides the AV matmul).
  Scores are computed transposed [k, q] in 3-k-tile PSUM chunks; one wide ACT
  exp per chunk evicts to SBUF bf16; causal masking is a single multiply per
  diagonal k-tile against a host-provided mask; AV accumulates [out^T | denom];
  normalization uses a DMA-reshaped reciprocal ([1,512] -> [128,4] so the DVE
  divides 4 elements per lane instead of 512) and a GPSIMD partition broadcast.
"""

import sys

for _p in ("/opt/trn_rl_repo",):
    if _p not in sys.path:
        sys.path.insert(0, _p)

import numpy as np
import ml_dtypes

BF16 = ml_dtypes.bfloat16

B, S, D = 4, 2048, 1024
H, HD = 16, 64
HPC = H // 2          # heads per core: 8
DPC = D // 2          # projection cols per core: 512
NCORES = 8
SCALE = 1.0 / np.sqrt(np.float32(HD))
CH = 3                # k-tiles per score chunk (3 PSUM banks, double buffered)

_compiled = None


def _chunks(nkt):
    out, s = [], 0
    while s < nkt:
        n = min(CH, nkt - s)
        out.append((s, n))
        s += n
    return out


def _build():
    import concourse.bacc as bacc
    import concourse.mybir as mybir
    import concourse.tile as tile

    f32 = mybir.dt.float32
    bf = mybir.dt.bfloat16
    Exp = mybir.ActivationFunctionType.Exp
    Copy = mybir.ActivationFunctionType.Copy

    nc = bacc.Bacc("TRN2", target_bir_lowering=False, debug=False)

    xtq = nc.dram_tensor("xtq", [D, S], bf, kind="ExternalInput")
    xtk = nc.dram_tensor("xtk", [D, S], bf, kind="ExternalInput")
    xtv = nc.dram_tensor("xtv", [D, S], bf, kind="ExternalInput")
    wq = nc.dram_tensor("wq", [D, DPC], bf, kind="ExternalInput")
    wk = nc.dram_tensor("wk", [D, DPC], bf, kind="ExternalInput")
    wv = nc.dram_tensor("wv", [D, DPC], bf, kind="ExternalInput")
    wo = nc.dram_tensor("wo", [DPC, D], bf, kind="ExternalInput")
    bq = nc.dram_tensor("bq", [1, DPC], bf, kind="ExternalInput")
    bk = nc.dram_tensor("bk", [1, DPC], bf, kind="ExternalInput")
    bv = nc.dram_tensor("bv", [1, DPC], bf, kind="ExternalInput")
    dmask = nc.dram_tensor("dmask", [128, 2048], bf, kind="ExternalInput")
    y = nc.dram_tensor("y", [S, D], f32, kind="ExternalOutput")

    NKD = D // 128        # 8 contraction tiles for projections
    NST = S // 128        # 16 seq tiles
    NSB = S // 512        # 4 seq blocks
    NHP = HPC // 2        # 4 head pairs / 128-wide col groups

    with tile.TileContext(nc) as tc:
        with (
            tc.tile_pool(name="consts", bufs=1) as consts,
            tc.tile_pool(name="wqp", bufs=NKD) as wqp,
            tc.tile_pool(name="wkp", bufs=NKD) as wkp,
            tc.tile_pool(name="wvp", bufs=NKD) as wvp,
            tc.tile_pool(name="wop", bufs=4) as wop,
            tc.tile_pool(name="xt", bufs=NKD) as xtp,
            tc.tile_pool(name="qt", bufs=NHP) as qtp,
            tc.tile_pool(name="kt", bufs=NHP) as ktp,
            tc.tile_pool(name="vp", bufs=NST) as vpool,
            tc.tile_pool(name="ex", bufs=4) as expool,
            tc.tile_pool(name="ot", bufs=NHP) as otp,
            tc.tile_pool(name="ys", bufs=3) as ysp,
            tc.tile_pool(name="rb", bufs=2) as rbp,
            tc.tile_pool(name="rc", bufs=2) as rcp,
            tc.tile_pool(name="ps", bufs=2, space="PSUM") as psp,
            tc.tile_pool(name="sc", bufs=2, space="PSUM") as scp,
        ):
            # constants
            dmt = consts.tile([128, 2048], bf, tag="dmt")
            nc.sync.dma_start(dmt[:], dmask.ap()[:])
            ones = consts.tile([1, 512], bf, tag="ones")
            nc.gpsimd.memset(ones[:], 1.0)
            bqt = consts.tile([1, DPC], bf, tag="bq")
            nc.sync.dma_start(bqt[:], bq.ap()[:])
            bkt = consts.tile([1, DPC], bf, tag="bk")
            nc.sync.dma_start(bkt[:], bk.ap()[:])
            bvt = consts.tile([1, DPC], bf, tag="bv")
            nc.sync.dma_start(bvt[:], bv.ap()[:])

            # weights
            wqt, wkt, wvt = [], [], []
            for kd in range(NKD):
                for lst, pool, t in ((wqt, wqp, wq), (wkt, wkp, wk), (wvt, wvp, wv)):
                    w = pool.tile([128, DPC], bf)
                    nc.sync.dma_start(w[:], t.ap()[kd * 128:(kd + 1) * 128, :])
                    lst.append(w)
            wot = []
            for hp in range(4):
                w = wop.tile([128, D], bf)
                nc.sync.dma_start(w[:], wo.ap()[hp * 128:(hp + 1) * 128, :])
                wot.append(w)

            # ---- V projection (natural layout), interleaved [8 heads x 65] + ones col
            vts = []
            xts = []
            for kd in range(NKD):
                xt = xtp.tile([128, S], bf)
                nc.sync.dma_start(xt[:], xtv.ap()[kd * 128:(kd + 1) * 128, :])
                xts.append(xt)
            for st in range(NST):
                vt = vpool.tile([128, HPC * 65], bf)
                vts.append(vt)
                ps = psp.tile([128, 512], f32, tag="ps")
                for kd in range(NKD):
                    nc.tensor.matmul(
                        ps[:],
                        xts[kd][:, st * 128:(st + 1) * 128],
                        wvt[kd][:],
                        start=(kd == 0), stop=False,
                    )
                nc.tensor.matmul(ps[:], ones[0:1, 0:128], bvt[0:1, :],
                                 start=False, stop=True)
                v3 = vt[:].rearrange("p (h c) -> p h c", h=HPC, c=65)
                nc.scalar.activation(
                    v3[:, :, 0:64],
                    ps[:].rearrange("p (h c) -> p h c", h=HPC, c=64),
                    Copy,
                )
                nc.gpsimd.memset(v3[:, :, 64:65], 1.0)

            # ---- QT / KT projections: QT[n, s] = sum_d W[d, n] * xT[d, s] + b[n]
            qts, kts = [], []
            for (src, wts, bias, dest_list, dest_pool) in (
                (xtq, wqt, bqt, qts, qtp),
                (xtk, wkt, bkt, kts, ktp),
            ):
                xts = []
                for kd in range(NKD):
                    xt = xtp.tile([128, S], bf)
                    nc.sync.dma_start(xt[:], src.ap()[kd * 128:(kd + 1) * 128, :])
                    xts.append(xt)
                for hp in range(NHP):
                    dt_tile = dest_pool.tile([128, S], bf)
                    dest_list.append(dt_tile)
                    for sb in range(NSB):
                        ps = psp.tile([128, 512], f32, tag="ps")
                        for kd in range(NKD):
                            nc.tensor.matmul(
                                ps[:],
                                wts[kd][:, hp * 128:(hp + 1) * 128],
                                xts[kd][:, sb * 512:(sb + 1) * 512],
                                start=(kd == 0), stop=False,
                            )
                        nc.tensor.matmul(
                            ps[:],
                            bias[0:1, hp * 128:(hp + 1) * 128],
                            ones[0:1, :],
                            start=False, stop=True,
                        )
                        nc.scalar.activation(
                            dt_tile[:, sb * 512:(sb + 1) * 512], ps[:], Copy)

            # ---- attention per head; scoresT [k, q] chunks of CH k-tiles
            ots = [otp.tile([128, S], bf, name=f"ot{i}", tag="ot") for i in range(NHP)]
            for h in range(HPC):
                hp, sub = h // 2, h % 2
                base = sub * 64
                qt_h = qts[hp][base:base + 64, :]
                kt_h = kts[hp][base:base + 64, :]
                for j in range(NSB):
                    av = psp.tile([128, 512], f32, tag="ps")
                    nkt = 4 * (j + 1)
                    for (c0, cn) in _chunks(nkt):
                        sc = scp.tile([128, CH * 512], f32)
                        for r in range(cn):
                            kti = c0 + r
                            nc.tensor.matmul(
                                sc[:, r * 512:(r + 1) * 512],
                                kt_h[:, kti * 128:(kti + 1) * 128],
                                qt_h[:, j * 512:(j + 1) * 512],
                                start=True, stop=True,
                            )
                        ex = expool.tile([128, CH * 512], bf)
                        nc.scalar.activation(
                            ex[:, 0:cn * 512], sc[:, 0:cn * 512], Exp,
                            scale=float(SCALE))
                        for r in range(cn):
                            kti = c0 + r
                            rr = kti - 4 * j
                            if rr >= 0:   # diagonal k-tile: causal mask multiply
                                nc.vector.tensor_mul(
                                    ex[:, r * 512:(r + 1) * 512],
                                    ex[:, r * 512:(r + 1) * 512],
                                    dmt[:, rr * 512:(rr + 1) * 512],
                                )
                        for r in range(cn):
                            kti = c0 + r
                            nc.tensor.matmul(
                                av[0:65, :],
                                vts[kti][:, h * 65:(h + 1) * 65],
                                ex[:, r * 512:(r + 1) * 512],
                                start=(kti == 0), stop=(kti == nkt - 1),
                            )
                    # normalize: denom row -> [128,4] reshape -> fast recip -> bcast
                    drow = rcp.tile([1, 512], f32, tag="drow")
                    nc.vector.tensor_copy(drow[:], av[64:65, :])
                    rsh = rcp.tile([128, 4], f32, tag="rsh")
                    nc.sync.dma_start(rsh[:], drow[:])
                    rr_t = rcp.tile([128, 4], f32, tag="rr")
                    nc.vector.reciprocal(rr_t[:], rsh[:])
                    rrow = rcp.tile([1, 512], f32, tag="rrow")
                    nc.sync.dma_start(rrow[:], rr_t[:])
                    rb = rbp.tile([64, 512], f32)
                    nc.gpsimd.partition_broadcast(rb[:], rrow[:], channels=64)
                    nc.vector.tensor_mul(
                        ots[hp][base:base + 64, j * 512:(j + 1) * 512],
                        av[0:64, :],
                        rb[:],
                    )

            # ---- output projection: y[s, e] = sum_d outT[d, s] * Wo[d, e]
            for st in range(NST):
                for eb in range(2):
                    ps = psp.tile([128, 512], f32, tag="ps")
                    for hp in range(NHP):
                        nc.tensor.matmul(
                            ps[:],
                            ots[hp][:, st * 128:(st + 1) * 128],
                            wot[hp][:, eb * 512:(eb + 1) * 512],
                            start=(hp == 0), stop=(hp == NHP - 1),
                        )
                    ys = ysp.tile([128, 512], f32)
                    nc.scalar.activation(ys[:], ps[:], Copy)
                    nc.sync.dma_start(
                        y.ap()[st * 128:(st + 1) * 128, eb * 512:(eb + 1) * 512],
                        ys[:],
                    )

    nc.compile()
    return nc


def _diag_mask():
    tri = np.triu(np.ones((128, 128), np.float32))  # mask[k,q]=1 iff k<=q
    m = np.ones((128, 2048), np.float32)
    for r in range(4):
        m[:, r * 512:r * 512 + r * 128] = 0.0
        m[:, r * 512 + r * 128:r * 512 + (r + 1) * 128] = tri
    return m.astype(BF16)


def _shard_inputs(q_in, k_in, v_in, Wq, bq, Wk, bk, Wv, bv, Wo, bo):
    dm = _diag_mask()
    in_maps = []
    for core in range(NCORES):
        b, g = core // 2, core % 2
        cs = slice(g * DPC, (g + 1) * DPC)
        in_maps.append({
            "xtq": np.ascontiguousarray(q_in[b].T).astype(BF16),
            "xtk": np.ascontiguousarray(k_in[b].T).astype(BF16),
            "xtv": np.ascontiguousarray(v_in[b].T).astype(BF16),
            "wq": Wq[:, cs].astype(BF16),
            "wk": Wk[:, cs].astype(BF16),
            "wv": Wv[:, cs].astype(BF16),
            "wo": np.ascontiguousarray(Wo[cs, :]).astype(BF16),
            "bq": bq[cs].reshape(1, DPC).astype(BF16),
            "bk": bk[cs].reshape(1, DPC).astype(BF16),
            "bv": bv[cs].reshape(1, DPC).astype(BF16),
            "dmask": dm,
        })
    return in_maps


def kernel(q_in, k_in, v_in, Wq, bq, Wk, bk, Wv, bv, Wo, bo, _trace=False):
    from concourse.bass_utils import run_bass_kernel_spmd

    global _compiled
    if _compiled is None:
        _compiled = _build()

    args = [np.asarray(a, np.float32) for a in
            (q_in, k_in, v_in, Wq, bq, Wk, bk, Wv, bv, Wo, bo)]
    in_maps = _shard_inputs(*args)
    res = run_bass_kernel_spmd(
        _compiled, in_maps, core_ids=list(range(NCORES)), trace=_trace,
    )
    bo_f = args[10]
    out = np.empty((B, S, D), np.float32)
    for b in range(B):
        out[b] = res.results[2 * b]["y"] + res.results[2 * b + 1]["y"] + bo_f
    if _trace:
        kernel.last_results = res
    return out


# revision 8
# speedup vs baseline: 1.4876x; 1.0541x over previous
"""Multi-head attention (B=4, S=2048, D=1024, H=16, causal) on 8 TRN2 NeuronCores.

Sharding: core i handles batch i//2 and head-group i%2 (8 heads / 512 projection
columns). Each core computes a partial output projection over its 512 rows of Wo;
the host sums the two partials per batch and adds bo. No device collectives.

Per-core dataflow (bf16 matmuls, fp32 softmax):
  QT/KT = W-stationary projections of pre-transposed x; V in natural layout with
  an interleaved ones column per head (softmax denominator rides the AV matmul).
  Scores are computed transposed [k, q] in 3-k-tile PSUM chunks; one wide ACT
  exp per chunk evicts to SBUF bf16; causal masking is a single multiply per
  diagonal k-tile against a host-provided mask; AV accumulates [out^T | denom];
  normalization uses a DMA-reshaped reciprocal ([1,512] -> [128,4] so the DVE
  divides 4 elements per lane instead of 512) and a GPSIMD partition broadcast.
"""

import sys

for _p in ("/opt/trn_rl_repo",):
    if _p not in sys.path:
        sys.path.insert(0, _p)

import numpy as np
import ml_dtypes

BF16 = ml_dtypes.bfloat16

B, S, D = 4, 2048, 1024
H, HD = 16, 64
HPC = H // 2          # heads per core: 8
DPC = D // 2          # projection cols per core: 512
NCORES = 8
SCALE = 1.0 / np.sqrt(np.float32(HD))
CH = 3                # k-tiles per score chunk (3 PSUM banks, double buffered)

_compiled = None


def _chunks(nkt):
    out, s = [], 0
    while s < nkt:
        n = min(CH, nkt - s)
        out.append((s, n))
        s += n
    return out


def _build():
    import concourse.bacc as bacc
    import concourse.mybir as mybir
    import concourse.tile as tile

    f32 = mybir.dt.float32
    bf = mybir.dt.bfloat16
    Exp = mybir.ActivationFunctionType.Exp
    Copy = mybir.ActivationFunctionType.Copy

    nc = bacc.Bacc("TRN2", target_bir_lowering=False, debug=False)

    xtq = nc.dram_tensor("xtq", [D, S], bf, kind="ExternalInput")
    xtk = nc.dram_tensor("xtk", [D, S], bf, kind="ExternalInput")
    xtv = nc.dram_tensor("xtv", [D, S], bf, kind="ExternalInput")
    wq = nc.dram_tensor("wq", [D, DPC], bf, kind="ExternalInput")
    wk = nc.dram_tensor("wk", [D, DPC], bf, kind="ExternalInput")
    wv = nc.dram_tensor("wv", [D, DPC], bf, kind="ExternalInput")
    wo = nc.dram_tensor("wo", [DPC, D], bf, kind="ExternalInput")
    bq = nc.dram_tensor("bq", [1, DPC], bf, kind="ExternalInput")
    bk = nc.dram_tensor("bk", [1, DPC], bf, kind="ExternalInput")
    bv = nc.dram_tensor("bv", [1, DPC], bf, kind="ExternalInput")
    dmask = nc.dram_tensor("dmask", [128, 2048], bf, kind="ExternalInput")
    y = nc.dram_tensor("y", [S, D], f32, kind="ExternalOutput")

    NKD = D // 128        # 8 contraction tiles for projections
    NST = S // 128        # 16 seq tiles
    NSB = S // 512        # 4 seq blocks
    NHP = HPC // 2        # 4 head pairs / 128-wide col groups

    with tile.TileContext(nc) as tc:
        with (
            tc.tile_pool(name="consts", bufs=1) as consts,
            tc.tile_pool(name="wqp", bufs=NKD) as wqp,
            tc.tile_pool(name="wkp", bufs=NKD) as wkp,
            tc.tile_pool(name="wvp", bufs=NKD) as wvp,
            tc.tile_pool(name="wop", bufs=4) as wop,
            tc.tile_pool(name="xt", bufs=2 * NKD) as xtp,
            tc.tile_pool(name="qt", bufs=NHP) as qtp,
            tc.tile_pool(name="kt", bufs=NHP) as ktp,
            tc.tile_pool(name="vp", bufs=NST) as vpool,
            tc.tile_pool(name="ex", bufs=3) as expool,
            tc.tile_pool(name="ot", bufs=NHP) as otp,
            tc.tile_pool(name="ys", bufs=2) as ysp,
            tc.tile_pool(name="rb", bufs=2) as rbp,
            tc.tile_pool(name="rc", bufs=2) as rcp,
            tc.tile_pool(name="ps", bufs=2, space="PSUM") as psp,
            tc.tile_pool(name="sc", bufs=2, space="PSUM") as scp,
        ):
            # constants
            dmt = consts.tile([128, 2048], bf, tag="dmt")
            nc.sync.dma_start(dmt[:], dmask.ap()[:])
            ones = consts.tile([1, 512], bf, tag="ones")
            nc.gpsimd.memset(ones[:], 1.0)
            bqt = consts.tile([1, DPC], bf, tag="bq")
            nc.sync.dma_start(bqt[:], bq.ap()[:])
            bkt = consts.tile([1, DPC], bf, tag="bk")
            nc.sync.dma_start(bkt[:], bk.ap()[:])
            bvt = consts.tile([1, DPC], bf, tag="bv")
            nc.sync.dma_start(bvt[:], bv.ap()[:])

            # weights
            wqt, wkt, wvt = [], [], []
            for kd in range(NKD):
                for lst, pool, t in ((wqt, wqp, wq), (wkt, wkp, wk), (wvt, wvp, wv)):
                    w = pool.tile([128, DPC], bf)
                    nc.sync.dma_start(w[:], t.ap()[kd * 128:(kd + 1) * 128, :])
                    lst.append(w)
            wot = []
            for hp in range(4):
                w = wop.tile([128, D], bf)
                nc.sync.dma_start(w[:], wo.ap()[hp * 128:(hp + 1) * 128, :])
                wot.append(w)

            # ---- V projection (natural layout), interleaved [8 heads x 65] + ones col
            vts = []
            xts = []
            for kd in range(NKD):
                xt = xtp.tile([128, S], bf, name=f"xt_v{kd}", tag="xt")
                nc.sync.dma_start(xt[:], xtv.ap()[kd * 128:(kd + 1) * 128, :])
                xts.append(xt)
            for st in range(NST):
                vt = vpool.tile([128, HPC * 65], bf)
                vts.append(vt)
                ps = psp.tile([128, 512], f32, tag="ps")
                for kd in range(NKD):
                    nc.tensor.matmul(
                        ps[:],
                        xts[kd][:, st * 128:(st + 1) * 128],
                        wvt[kd][:],
                        start=(kd == 0), stop=False,
                    )
                nc.tensor.matmul(ps[:], ones[0:1, 0:128], bvt[0:1, :],
                                 start=False, stop=True)
                v3 = vt[:].rearrange("p (h c) -> p h c", h=HPC, c=65)
                nc.scalar.activation(
                    v3[:, :, 0:64],
                    ps[:].rearrange("p (h c) -> p h c", h=HPC, c=64),
                    Copy,
                )
                nc.gpsimd.memset(v3[:, :, 64:65], 1.0)

            # ---- QT / KT projections. hp=0 is emitted up front; hp=1..3 are
            # queued as fine-grained filler steps woven between attention chunks
            # so the tensor engine never idles below the HAM busy threshold
            # while ACT paces the exp pipeline.
            qts, kts = [], []
            xtq_ts, xtk_ts = [], []
            for (src_t, xlist) in ((xtq, xtq_ts), (xtk, xtk_ts)):
                for kd in range(NKD):
                    xt = xtp.tile([128, S], bf, name=f"xt_{src_t.name}{kd}", tag="xt")
                    nc.sync.dma_start(xt[:], src_t.ap()[kd * 128:(kd + 1) * 128, :])
                    xlist.append(xt)
            for pool, lst, nm in ((qtp, qts, "qt"), (ktp, kts, "kt")):
                for hp in range(NHP):
                    lst.append(pool.tile([128, S], bf, name=f"{nm}{hp}", tag=nm))

            def proj_group_steps(xts, wts, bias, dest, hp, sb, on_act):
                ps_box = []
                def mk_mm(kd):
                    def step():
                        if kd == 0:
                            ps_box.append(psp.tile([128, 512], f32, name="psq", tag="ps"))
                        nc.tensor.matmul(
                            ps_box[0][:],
                            wts[kd][:, hp * 128:(hp + 1) * 128],
                            xts[kd][:, sb * 512:(sb + 1) * 512],
                            start=(kd == 0), stop=False,
                        )
                    return step
                def bias_step():
                    nc.tensor.matmul(
                        ps_box[0][:],
                        bias[0:1, hp * 128:(hp + 1) * 128],
                        ones[0:1, :],
                        start=False, stop=True,
                    )
                def evict():
                    dst = dest[:, sb * 512:(sb + 1) * 512]
                    if on_act:
                        nc.scalar.activation(dst, ps_box[0][:], Copy)
                    else:
                        nc.vector.tensor_copy(dst, ps_box[0][:])
                return [mk_mm(kd) for kd in range(NKD)] + [bias_step, evict]

            # hp=0 up front (attention for heads 0/1 needs it)
            for (xts, wts, bias, dest) in (
                (xtq_ts, wqt, bqt, qts[0]),
                (xtk_ts, wkt, bkt, kts[0]),
            ):
                for sb in range(NSB):
                    for step in proj_group_steps(xts, wts, bias, dest, 0, sb, True):
                        step()

            # filler queue: hp=1..3 (evictions on DVE: ACT paces exp in attention)
            filler = []
            for hp in range(1, NHP):
                for (xts, wts, bias, dest) in (
                    (xtq_ts, wqt, bqt, qts[hp]),
                    (xtk_ts, wkt, bkt, kts[hp]),
                ):
                    for sb in range(NSB):
                        filler.extend(
                            proj_group_steps(xts, wts, bias, dest, hp, sb, False))
            filler.reverse()  # pop() from the front

            # ---- attention per head; scoresT [k, q] chunks of CH k-tiles
            n_chunks_left = sum(len(_chunks(4 * (j + 1))) for j in range(NSB)) * HPC
            ots = [otp.tile([128, S], bf, name=f"ot{i}", tag="ot") for i in range(NHP)]
            for h in range(HPC):
                hp, sub = h // 2, h % 2
                base = sub * 64
                qt_h = qts[hp][base:base + 64, :]
                kt_h = kts[hp][base:base + 64, :]
                for j in range(NSB):
                    av = psp.tile([128, 512], f32, tag="ps")
                    nkt = 4 * (j + 1)
                    for (c0, cn) in _chunks(nkt):
                        sc = scp.tile([128, CH * 512], f32)
                        for r in range(cn):
                            kti = c0 + r
                            nc.tensor.matmul(
                                sc[:, r * 512:(r + 1) * 512],
                                kt_h[:, kti * 128:(kti + 1) * 128],
                                qt_h[:, j * 512:(j + 1) * 512],
                                start=True, stop=True,
                            )
                        ex = expool.tile([128, CH * 512], bf)
                        nc.scalar.activation(
                            ex[:, 0:cn * 512], sc[:, 0:cn * 512], Exp,
                            scale=float(SCALE))
                        for r in range(cn):
                            kti = c0 + r
                            rr = kti - 4 * j
                            if rr >= 0:   # diagonal k-tile: causal mask multiply
                                nc.vector.tensor_mul(
                                    ex[:, r * 512:(r + 1) * 512],
                                    ex[:, r * 512:(r + 1) * 512],
                                    dmt[:, rr * 512:(rr + 1) * 512],
                                )
                        for r in range(cn):
                            kti = c0 + r
                            nc.tensor.matmul(
                                av[0:65, :],
                                vts[kti][:, h * 65:(h + 1) * 65],
                                ex[:, r * 512:(r + 1) * 512],
                                start=(kti == 0), stop=(kti == nkt - 1),
                            )
                        # weave in projection filler to keep PE dense
                        if filler and h < 6:
                            import math as _m
                            want = max(1, -(-len(filler) // max(1, n_chunks_left)))
                            for _ in range(min(want + 1, len(filler))):
                                filler.pop()()
                        n_chunks_left -= 1
                    # normalize: denom -> [128,4] reshape -> fast recip -> bcast
                    drow = rcp.tile([1, 512], f32, tag="drow")
                    nc.vector.tensor_copy(drow[:], av[64:65, :])
                    rsh = rcp.tile([128, 4], f32, tag="rsh")
                    nc.sync.dma_start(rsh[:], drow[:])
                    rr_t = rcp.tile([128, 4], f32, tag="rr")
                    nc.vector.reciprocal(rr_t[:], rsh[:])
                    rrow = rcp.tile([1, 512], f32, tag="rrow")
                    nc.sync.dma_start(rrow[:], rr_t[:])
                    rb = rbp.tile([64, 512], f32)
                    nc.gpsimd.partition_broadcast(rb[:], rrow[:], channels=64)
                    nc.vector.tensor_mul(
                        ots[hp][base:base + 64, j * 512:(j + 1) * 512],
                        av[0:64, :],
                        rb[:],
                    )
            while filler:
                filler.pop()()

            # ---- output projection: y[s, e] = sum_d outT[d, s] * Wo[d, e]
            for st in range(NST):
                for eb in range(2):
                    ps = psp.tile([128, 512], f32, tag="ps")
                    for hp in range(NHP):
                        nc.tensor.matmul(
                            ps[:],
                            ots[hp][:, st * 128:(st + 1) * 128],
                            wot[hp][:, eb * 512:(eb + 1) * 512],
                            start=(hp == 0), stop=(hp == NHP - 1),
                        )
                    ys = ysp.tile([128, 512], f32)
                    nc.scalar.activation(ys[:], ps[:], Copy)
                    nc.sync.dma_start(
                        y.ap()[st * 128:(st + 1) * 128, eb * 512:(eb + 1) * 512],
                        ys[:],
                    )

    nc.compile()
    return nc


def _diag_mask():
    tri = np.triu(np.ones((128, 128), np.float32))  # mask[k,q]=1 iff k<=q
    m = np.ones((128, 2048), np.float32)
    for r in range(4):
        m[:, r * 512:r * 512 + r * 128] = 0.0
        m[:, r * 512 + r * 128:r * 512 + (r + 1) * 128] = tri
    return m.astype(BF16)


def _shard_inputs(q_in, k_in, v_in, Wq, bq, Wk, bk, Wv, bv, Wo, bo):
    dm = _diag_mask()
    in_maps = []
    for core in range(NCORES):
        b, g = core // 2, core % 2
        cs = slice(g * DPC, (g + 1) * DPC)
        in_maps.append({
            "xtq": np.ascontiguousarray(q_in[b].T).astype(BF16),
            "xtk": np.ascontiguousarray(k_in[b].T).astype(BF16),
            "xtv": np.ascontiguousarray(v_in[b].T).astype(BF16),
            "wq": Wq[:, cs].astype(BF16),
            "wk": Wk[:, cs].astype(BF16),
            "wv": Wv[:, cs].astype(BF16),
            "wo": np.ascontiguousarray(Wo[cs, :]).astype(BF16),
            "bq": bq[cs].reshape(1, DPC).astype(BF16),
            "bk": bk[cs].reshape(1, DPC).astype(BF16),
            "bv": bv[cs].reshape(1, DPC).astype(BF16),
            "dmask": dm,
        })
    return in_maps


def kernel(q_in, k_in, v_in, Wq, bq, Wk, bk, Wv, bv, Wo, bo, _trace=False):
    from concourse.bass_utils import run_bass_kernel_spmd

    global _compiled
    if _compiled is None:
        _compiled = _build()

    args = [np.asarray(a, np.float32) for a in
            (q_in, k_in, v_in, Wq, bq, Wk, bk, Wv, bv, Wo, bo)]
    in_maps = _shard_inputs(*args)
    res = run_bass_kernel_spmd(
        _compiled, in_maps, core_ids=list(range(NCORES)), trace=_trace,
    )
    bo_f = args[10]
    out = np.empty((B, S, D), np.float32)
    for b in range(B):
        out[b] = res.results[2 * b]["y"] + res.results[2 * b + 1]["y"] + bo_f
    if _trace:
        kernel.last_results = res
    return out


# revision 12
# speedup vs baseline: 1.6281x; 1.0944x over previous
"""Multi-head attention (B=4, S=2048, D=1024, H=16, causal) on 8 TRN2 NeuronCores.

Sharding: core i handles batch i//2 and head-group i%2 (8 heads / 512 projection
columns). Each core computes a partial output projection over its 512 rows of Wo;
the host sums the two partials per batch and adds bo. No device collectives.

Per-core dataflow (bf16 matmuls, fp32 softmax):
  QT/KT = W-stationary projections of pre-transposed x; V in natural layout with
  an interleaved ones column per head (softmax denominator rides the AV matmul).
  Scores are computed transposed [k, q] in 3-k-tile PSUM chunks; one wide ACT
  exp per chunk evicts to SBUF bf16; causal masking is a single multiply per
  diagonal k-tile against a host-provided mask; AV accumulates [out^T | denom];
  normalization uses a DMA-reshaped reciprocal ([1,512] -> [128,4] so the DVE
  divides 4 elements per lane instead of 512) and a GPSIMD partition broadcast.
"""

import sys

for _p in ("/opt/trn_rl_repo",):
    if _p not in sys.path:
        sys.path.insert(0, _p)

import numpy as np
import ml_dtypes

BF16 = ml_dtypes.bfloat16

B, S, D = 4, 2048, 1024
H, HD = 16, 64
HPC = H // 2          # heads per core: 8
DPC = D // 2          # projection cols per core: 512
NCORES = 8
SCALE = 1.0 / np.sqrt(np.float32(HD))
CH = 3                # k-tiles per score chunk (3 PSUM banks, double buffered)

_compiled = None


def _chunks(nkt):
    out, s = [], 0
    while s < nkt:
        n = min(CH, nkt - s)
        out.append((s, n))
        s += n
    return out


def _build():
    import concourse.bacc as bacc
    import concourse.mybir as mybir
    import concourse.tile as tile

    f32 = mybir.dt.float32
    bf = mybir.dt.bfloat16
    Exp = mybir.ActivationFunctionType.Exp
    Copy = mybir.ActivationFunctionType.Copy

    nc = bacc.Bacc("TRN2", target_bir_lowering=False, debug=False)

    xtq = nc.dram_tensor("xtq", [D, S], bf, kind="ExternalInput")
    xtk = nc.dram_tensor("xtk", [D, S], bf, kind="ExternalInput")
    xtv = nc.dram_tensor("xtv", [D, S], bf, kind="ExternalInput")
    wq = nc.dram_tensor("wq", [D, DPC], bf, kind="ExternalInput")
    wk = nc.dram_tensor("wk", [D, DPC], bf, kind="ExternalInput")
    wv = nc.dram_tensor("wv", [D, DPC], bf, kind="ExternalInput")
    wo = nc.dram_tensor("wo", [DPC, D], bf, kind="ExternalInput")
    bq = nc.dram_tensor("bq", [1, DPC], bf, kind="ExternalInput")
    bk = nc.dram_tensor("bk", [1, DPC], bf, kind="ExternalInput")
    bv = nc.dram_tensor("bv", [1, DPC], bf, kind="ExternalInput")
    dmask = nc.dram_tensor("dmask", [128, 2048], bf, kind="ExternalInput")
    y = nc.dram_tensor("y", [S, D], f32, kind="ExternalOutput")

    NKD = D // 128        # 8 contraction tiles for projections
    NST = S // 128        # 16 seq tiles
    NSB = S // 512        # 4 seq blocks
    NHP = HPC // 2        # 4 head pairs / 128-wide col groups

    with tile.TileContext(nc) as tc:
        with (
            tc.tile_pool(name="consts", bufs=1) as consts,
            tc.tile_pool(name="wqp", bufs=NKD) as wqp,
            tc.tile_pool(name="wkp", bufs=NKD) as wkp,
            tc.tile_pool(name="wvp", bufs=NKD) as wvp,
            tc.tile_pool(name="wop", bufs=4) as wop,
            tc.tile_pool(name="xt", bufs=2 * NKD) as xtp,
            tc.tile_pool(name="qt", bufs=NHP) as qtp,
            tc.tile_pool(name="kt", bufs=NHP) as ktp,
            tc.tile_pool(name="vp", bufs=NST) as vpool,
            tc.tile_pool(name="ex", bufs=3) as expool,
            tc.tile_pool(name="ot", bufs=NHP) as otp,
            tc.tile_pool(name="ys", bufs=2) as ysp,
            tc.tile_pool(name="rb", bufs=2) as rbp,
            tc.tile_pool(name="rc", bufs=1) as rcp,
            tc.tile_pool(name="ps", bufs=2, space="PSUM") as psp,
            tc.tile_pool(name="sc", bufs=2, space="PSUM") as scp,
        ):
            # constants
            dmt = consts.tile([128, 2048], bf, tag="dmt")
            nc.sync.dma_start(dmt[:], dmask.ap()[:])
            ones = consts.tile([1, 512], bf, tag="ones")
            nc.gpsimd.memset(ones[:], 1.0)
            bqt = consts.tile([1, DPC], bf, tag="bq")
            nc.sync.dma_start(bqt[:], bq.ap()[:])
            bkt = consts.tile([1, DPC], bf, tag="bk")
            nc.sync.dma_start(bkt[:], bk.ap()[:])
            bvt = consts.tile([1, DPC], bf, tag="bv")
            nc.sync.dma_start(bvt[:], bv.ap()[:])

            # weights: wv + xtv queued first so the first V matmul starts ASAP
            wvt = []
            for kd in range(NKD):
                w = wvp.tile([128, DPC], bf, name=f"wv{kd}", tag="wv")
                nc.sync.dma_start(w[:], wv.ap()[kd * 128:(kd + 1) * 128, :])
                wvt.append(w)
            vts = []
            xts = []
            for kd in range(NKD):
                xt = xtp.tile([128, S], bf, name=f"xt_v{kd}", tag="xt")
                nc.sync.dma_start(xt[:], xtv.ap()[kd * 128:(kd + 1) * 128, :])
                xts.append(xt)
            wqt, wkt = [], []
            for kd in range(NKD):
                for lst, pool, t in ((wqt, wqp, wq), (wkt, wkp, wk)):
                    w = pool.tile([128, DPC], bf)
                    nc.sync.dma_start(w[:], t.ap()[kd * 128:(kd + 1) * 128, :])
                    lst.append(w)
            wot = []
            for hp in range(4):
                w = wop.tile([128, D], bf)
                nc.sync.dma_start(w[:], wo.ap()[hp * 128:(hp + 1) * 128, :])
                wot.append(w)

            # ---- V projection (natural layout), interleaved [8 heads x 65] + ones col
            for st in range(NST):
                vt = vpool.tile([128, HPC * 65], bf)
                vts.append(vt)
                ps = psp.tile([128, 512], f32, tag="ps")
                for kd in range(NKD):
                    nc.tensor.matmul(
                        ps[:],
                        xts[kd][:, st * 128:(st + 1) * 128],
                        wvt[kd][:],
                        start=(kd == 0), stop=False,
                    )
                nc.tensor.matmul(ps[:], ones[0:1, 0:128], bvt[0:1, :],
                                 start=False, stop=True)
                v3 = vt[:].rearrange("p (h c) -> p h c", h=HPC, c=65)
                nc.scalar.activation(
                    v3[:, :, 0:64],
                    ps[:].rearrange("p (h c) -> p h c", h=HPC, c=64),
                    Copy,
                )
                nc.gpsimd.memset(v3[:, :, 64:65], 1.0)

            # ---- QT / KT projections. hp=0 is emitted up front; hp=1..3 are
            # queued as fine-grained filler steps woven between attention chunks
            # so the tensor engine never idles below the HAM busy threshold
            # while ACT paces the exp pipeline.
            qts, kts = [], []
            xtq_ts, xtk_ts = [], []
            for (src_t, xlist) in ((xtq, xtq_ts), (xtk, xtk_ts)):
                for kd in range(NKD):
                    xt = xtp.tile([128, S], bf, name=f"xt_{src_t.name}{kd}", tag="xt")
                    nc.sync.dma_start(xt[:], src_t.ap()[kd * 128:(kd + 1) * 128, :])
                    xlist.append(xt)
            for pool, lst, nm in ((qtp, qts, "qt"), (ktp, kts, "kt")):
                for hp in range(NHP):
                    lst.append(pool.tile([128, S], bf, name=f"{nm}{hp}", tag=nm))

            def proj_group_steps(xts, wts, bias, dest, hp, sb, on_act):
                ps_box = []
                def mk_mm(kd):
                    def step():
                        if kd == 0:
                            ps_box.append(psp.tile([128, 512], f32, name="psq", tag="ps"))
                        nc.tensor.matmul(
                            ps_box[0][:],
                            wts[kd][:, hp * 128:(hp + 1) * 128],
                            xts[kd][:, sb * 512:(sb + 1) * 512],
                            start=(kd == 0), stop=False,
                        )
                    return step
                def bias_step():
                    nc.tensor.matmul(
                        ps_box[0][:],
                        bias[0:1, hp * 128:(hp + 1) * 128],
                        ones[0:1, :],
                        start=False, stop=True,
                    )
                def evict():
                    dst = dest[:, sb * 512:(sb + 1) * 512]
                    if on_act:
                        nc.scalar.activation(dst, ps_box[0][:], Copy)
                    else:
                        nc.vector.tensor_copy(dst, ps_box[0][:])
                return [mk_mm(kd) for kd in range(NKD)] + [bias_step, evict]

            # hp=0 up front (attention for heads 0/1 needs it)
            for (xts, wts, bias, dest) in (
                (xtq_ts, wqt, bqt, qts[0]),
                (xtk_ts, wkt, bkt, kts[0]),
            ):
                for sb in range(NSB):
                    for step in proj_group_steps(xts, wts, bias, dest, 0, sb, True):
                        step()

            # filler queue: hp=1..3 (evictions on DVE: ACT paces exp in attention)
            filler = []
            for hp in range(1, NHP):
                for (xts, wts, bias, dest) in (
                    (xtq_ts, wqt, bqt, qts[hp]),
                    (xtk_ts, wkt, bkt, kts[hp]),
                ):
                    for sb in range(NSB):
                        filler.extend(
                            proj_group_steps(xts, wts, bias, dest, hp, sb, False))
            filler.reverse()  # pop() from the front

            # ---- attention per head; scoresT [k, q] chunks of CH k-tiles
            n_chunks_left = sum(len(_chunks(4 * (j + 1))) for j in range(NSB)) * 6
            ots = [otp.tile([128, S], bf, name=f"ot{i}", tag="ot") for i in range(NHP)]
            for h in range(HPC):
                hp, sub = h // 2, h % 2
                base = sub * 64
                qt_h = qts[hp][base:base + 64, :]
                kt_h = kts[hp][base:base + 64, :]
                for j in range(NSB):
                    av = psp.tile([128, 512], f32, tag="ps")
                    nkt = 4 * (j + 1)
                    for (c0, cn) in _chunks(nkt):
                        sc = scp.tile([128, CH * 512], f32)
                        for r in range(cn):
                            kti = c0 + r
                            nc.tensor.matmul(
                                sc[:, r * 512:(r + 1) * 512],
                                kt_h[:, kti * 128:(kti + 1) * 128],
                                qt_h[:, j * 512:(j + 1) * 512],
                                start=True, stop=True,
                            )
                        ex = expool.tile([128, CH * 512], bf)
                        nc.scalar.activation(
                            ex[:, 0:cn * 512], sc[:, 0:cn * 512], Exp,
                            scale=float(SCALE))
                        for r in range(cn):
                            kti = c0 + r
                            rr = kti - 4 * j
                            if rr >= 0:   # diagonal k-tile: causal mask multiply
                                nc.vector.tensor_mul(
                                    ex[:, r * 512:(r + 1) * 512],
                                    ex[:, r * 512:(r + 1) * 512],
                                    dmt[:, rr * 512:(rr + 1) * 512],
                                )
                        for r in range(cn):
                            kti = c0 + r
                            nc.tensor.matmul(
                                av[0:65, :],
                                vts[kti][:, h * 65:(h + 1) * 65],
                                ex[:, r * 512:(r + 1) * 512],
                                start=(kti == 0), stop=(kti == nkt - 1),
                            )
                        # weave in projection filler to keep PE dense
                        if filler and h < 6:
                            want = max(1, -(-len(filler) // max(1, n_chunks_left)))
                            for _ in range(min(want, len(filler))):
                                filler.pop()()
                        if h < 6:
                            n_chunks_left -= 1
                    # evict av to SBUF immediately (frees the PSUM slot), then
                    # normalize: denom -> [128,4] reshape -> fast recip -> bcast
                    avs = ysp.tile([65, 512], f32, name="avs", tag="ys")
                    nc.vector.tensor_copy(avs[:], av[0:65, :])
                    rsh = rcp.tile([128, 4], f32, tag="rsh")
                    nc.sync.dma_start(rsh[:], avs[64:65, :])
                    rr_t = rcp.tile([128, 4], f32, tag="rr")
                    nc.vector.reciprocal(rr_t[:], rsh[:])
                    rrow = rcp.tile([1, 512], f32, tag="rrow")
                    nc.sync.dma_start(rrow[:], rr_t[:])
                    rb = rbp.tile([64, 512], f32, tag="rb")
                    nc.gpsimd.partition_broadcast(rb[:], rrow[:], channels=64)
                    nc.vector.tensor_mul(
                        ots[hp][base:base + 64, j * 512:(j + 1) * 512],
                        avs[0:64, :],
                        rb[:],
                    )
            while filler:
                filler.pop()()

            # ---- output projection: y[s, e] = sum_d outT[d, s] * Wo[d, e]
            for st in range(NST):
                for eb in range(2):
                    ps = psp.tile([128, 512], f32, tag="ps")
                    for hp in range(NHP):
                        nc.tensor.matmul(
                            ps[:],
                            ots[hp][:, st * 128:(st + 1) * 128],
                            wot[hp][:, eb * 512:(eb + 1) * 512],
                            start=(hp == 0), stop=(hp == NHP - 1),
                        )
                    ys = ysp.tile([128, 512], f32)
                    nc.scalar.activation(ys[:], ps[:], Copy)
                    nc.sync.dma_start(
                        y.ap()[st * 128:(st + 1) * 128, eb * 512:(eb + 1) * 512],
                        ys[:],
                    )

    nc.compile()
    return nc


def _diag_mask():
    tri = np.triu(np.ones((128, 128), np.float32))  # mask[k,q]=1 iff k<=q
    m = np.ones((128, 2048), np.float32)
    for r in range(4):
        m[:, r * 512:r * 512 + r * 128] = 0.0
        m[:, r * 512 + r * 128:r * 512 + (r + 1) * 128] = tri
    return m.astype(BF16)


def _shard_inputs(q_in, k_in, v_in, Wq, bq, Wk, bk, Wv, bv, Wo, bo):
    dm = _diag_mask()
    in_maps = []
    for core in range(NCORES):
        b, g = core // 2, core % 2
        cs = slice(g * DPC, (g + 1) * DPC)
        in_maps.append({
            "xtq": np.ascontiguousarray(q_in[b].T).astype(BF16),
            "xtk": np.ascontiguousarray(k_in[b].T).astype(BF16),
            "xtv": np.ascontiguousarray(v_in[b].T).astype(BF16),
            "wq": Wq[:, cs].astype(BF16),
            "wk": Wk[:, cs].astype(BF16),
            "wv": Wv[:, cs].astype(BF16),
            "wo": np.ascontiguousarray(Wo[cs, :]).astype(BF16),
            "bq": bq[cs].reshape(1, DPC).astype(BF16),
            "bk": bk[cs].reshape(1, DPC).astype(BF16),
            "bv": bv[cs].reshape(1, DPC).astype(BF16),
            "dmask": dm,
        })
    return in_maps


def kernel(q_in, k_in, v_in, Wq, bq, Wk, bk, Wv, bv, Wo, bo, _trace=False):
    from concourse.bass_utils import run_bass_kernel_spmd

    global _compiled
    if _compiled is None:
        _compiled = _build()

    args = [np.asarray(a, np.float32) for a in
            (q_in, k_in, v_in, Wq, bq, Wk, bk, Wv, bv, Wo, bo)]
    in_maps = _shard_inputs(*args)
    res = run_bass_kernel_spmd(
        _compiled, in_maps, core_ids=list(range(NCORES)), trace=_trace,
    )
    bo_f = args[10]
    out = np.empty((B, S, D), np.float32)
    for b in range(B):
        out[b] = res.results[2 * b]["y"] + res.results[2 * b + 1]["y"] + bo_f
    if _trace:
        kernel.last_results = res
    return out
